# revision 7
# baseline (speedup 1.0000x reference)
"""Trainium2 Bass kernel for BoundaryLoss.

loss = mean over pixels of BCE(pred_b, tgt_b) where pred_b/tgt_b are 0/1
Sobel-boundary maps of sigmoid(logits) / targets. Since both maps are
binary, the clamped BCE reduces exactly to 100 * mean(pred_b XOR tgt_b).

Strategy (pure data parallel over batch, 2 samples -> 8 images per core):
  - logits loaded fp32 (HWDGE); sigmoid fp32->fp16 on ScalarE
  - targets loaded via SWDGE cast-DMA fp32->fp16 (no engine cost)
  - pred/tgt fp16 chunks interleaved in one tile so each block-set shares
    a single 4-bank PSUM tile [gx_p|gx_t|gy_p|gy_t]
  - full 2D Sobel conv on TensorE fp16: vertical band-matrix matmuls with
    column-shifted moving operands accumulating in PSUM (gx: 2 matmuls,
    gy: 3 matmuls, per src) per 126-row block
  - per-set evacuation alternates two schemes to balance ScalarE/VectorE:
      OptA: ScalarE squares all 4 quadrants (1 inst, 2048 wide); VectorE
            margins (stt add,add), prod, and is_lt count
      OptB: ScalarE squares only the gx pair (1024); VectorE computes
            negated gy^2 straight from PSUM via stt (gy*-1)*gy, booleans
            via (sq_x-0.25) is_gt nsq_y, count via not_equal + accum_out
  - per-partition counts accumulate into one counts column per set;
    final sum on host.
Each 512-row image = 4 main blocks of 126 output rows (input 128 rows
incl. 1-row halo) + an 8-row leftover; leftovers of all 8 images are
batched into one block-diagonal matmul set.
"""
import os
import numpy as np

import concourse.bass as bass
import concourse.tile as tile
from concourse import bacc, mybir
from concourse.bass_utils import run_bass_kernel_spmd

F32 = mybir.dt.float32
F16 = mybir.dt.float16
AF = mybir.ActivationFunctionType
OP = mybir.AluOpType

B, C, H, W = 16, 4, 512, 512
N_CORES = 8
BPC = B // N_CORES          # batch entries per core
N_IMG = BPC * C             # images per core
MAIN_BLOCKS = [(0, 0, 127, 126), (126, 125, 128, 126),
               (252, 251, 128, 126), (378, 377, 128, 126)]
LEFT_IN, LEFT_OUT, LEFT_K, LEFT_M = 503, 504, 9, 8
N_SETS = N_IMG * len(MAIN_BLOCKS) + 1   # 33 count columns
CW = 514                                 # block chunk width incl pad cols
# fraction of sets routed to OptB (VectorE-heavy evacuation): set i uses
# OptB iff (i * BETA_NUM) % BETA_DEN < BETA_NUM
BETA_NUM = int(os.environ.get("BASS_BETA_NUM", "1"))
BETA_DEN = int(os.environ.get("BASS_BETA_DEN", "2"))


def _use_opt_b(set_idx):
    return (set_idx * BETA_NUM) % BETA_DEN < BETA_NUM


# ---------------------------------------------------------------- bands
def _band_pair(in_rows, out_rows):
    K, M = len(in_rows), len(out_rows)
    vs = np.zeros((K, M), np.float32)
    vd = np.zeros((K, M), np.float32)
    for k, ir in enumerate(in_rows):
        for m, orow in enumerate(out_rows):
            d = ir - orow
            if d == 0:
                vs[k, m] = 2.0
            elif abs(d) == 1:
                vs[k, m] = 1.0
                vd[k, m] = float(d)
    return vs, vd


def _build_band_tensor():
    """Stack all band matrices into one [128, total_cols] array.
    offsets[(key, wname)] = (col, K, M)."""
    specs = {}
    specs['b0'] = _band_pair(range(0, 127), range(0, 126))
    specs['int'] = _band_pair(range(125, 253), range(126, 252))
    K, M = LEFT_K * N_IMG, LEFT_M * N_IMG
    vs = np.zeros((K, M), np.float32)
    vd = np.zeros((K, M), np.float32)
    svs, svd = _band_pair(range(LEFT_IN, 512), range(LEFT_OUT, 512))
    for i in range(N_IMG):
        vs[i*LEFT_K:(i+1)*LEFT_K, i*LEFT_M:(i+1)*LEFT_M] = svs
        vd[i*LEFT_K:(i+1)*LEFT_K, i*LEFT_M:(i+1)*LEFT_M] = svd
    specs['left'] = (vs, vd)

    cols = []
    offsets = {}
    col = 0
    for key, (vs, vd) in specs.items():
        for wname, wmat in (("vs", vs), ("vsn", -vs), ("vd2", 2.0*vd), ("vd", vd)):
            K, M = wmat.shape
            buf = np.zeros((128, M), np.float32)
            buf[:K, :] = wmat
            cols.append(buf)
            offsets[(key, wname)] = (col, K, M)
            col += M
    return np.concatenate(cols, axis=1), offsets


_BANDS, _BOFF = _build_band_tensor()
BANDW = _BANDS.shape[1]


# ---------------------------------------------------------------- kernel
_PIECES = os.environ.get("BASS_KERNEL_PIECES", "full")


def _emit_tail(nc, wsb, counts_sb, set_idx, src, col0, K, M,
               band_key, psum_pool, sq_pool, m_pool, prod_pool, nsq_pool,
               bias0):
    """Matmuls + squares + margins + xor count for one block-set.
    src: fp16 tile viewed [128, nchunk, 2, 514]; col0 selects the chunk.
    Data at cols [1, 513) of each (chunk, src) slot, zero pads at 0/513."""
    if _PIECES in ("io", "dma"):
        return

    def wap(wname):
        col, kk, mm = _BOFF[(band_key, wname)]
        assert kk == K and mm == M
        return wsb[0:K, col:col + M]

    ps = psum_pool.tile([128, 2048], F32, tag="ps")
    # quadrant layout: [gx_p | gx_t | gy_p | gy_t], 512 fp32 = 1 bank each
    for half in (0, 1):
        mv = lambda s: src[0:K, col0, half, s:s+512]
        gx = ps[0:M, half*512:(half+1)*512]
        nc.tensor.matmul(gx, wap("vs"), mv(2), start=True, stop=False)
        nc.tensor.matmul(gx, wap("vsn"), mv(0), start=False, stop=True)
    for half in (0, 1):
        mv = lambda s: src[0:K, col0, half, s:s+512]
        gy = ps[0:M, 1024+half*512:1024+(half+1)*512]
        nc.tensor.matmul(gy, wap("vd2"), mv(1), start=True, stop=False)
        nc.tensor.matmul(gy, wap("vd"), mv(0), start=False, stop=False)
        nc.tensor.matmul(gy, wap("vd"), mv(2), start=False, stop=True)

    if _PIECES == "conv":
        return

    sq = sq_pool.tile([128, 2048], F16, tag="sq")
    m = m_pool.tile([128, 1024], F16, tag="m")
    if not _use_opt_b(set_idx):
        # OptA: ScalarE-heavy
        nc.scalar.activation(sq[0:M, :], ps[0:M, :], AF.Square,
                             bias=bias0[0:M, 0:1])
        # m = (sq_x - 0.25) + sq_y
        nc.vector.scalar_tensor_tensor(m[0:M, 0:512], sq[0:M, 0:512], -0.25,
                                       sq[0:M, 1024:1536], OP.add, OP.add)
        nc.vector.scalar_tensor_tensor(m[0:M, 512:1024], sq[0:M, 512:1024],
                                       -0.25, sq[0:M, 1536:2048],
                                       OP.add, OP.add)
        prod = prod_pool.tile([128, 512], F16, tag="prod")
        nc.vector.tensor_tensor(prod[0:M, :], m[0:M, 0:512], m[0:M, 512:1024],
                                OP.mult)
        ind = prod_pool.tile([128, 512], F16, tag="ind")
        # out = (prod < 0); accum_out = per-partition sum (op1 = reduce op)
        nc.vector.tensor_scalar(ind[0:M, :], prod[0:M, :], 0.0, None,
                                OP.is_lt, OP.add,
                                accum_out=counts_sb[0:M, set_idx:set_idx+1])
    else:
        # OptB: VectorE-heavy; gy pair copied out of PSUM on VectorE and
        # squared negated there (stt cannot read PSUM twice), so booleans
        # come out of one stt each
        nc.scalar.activation(sq[0:M, 0:1024], ps[0:M, 0:1024], AF.Square,
                             bias=bias0[0:M, 0:1])
        gyc = nsq_pool.tile([128, 1024], F16, tag="gyc")
        nc.vector.tensor_copy(gyc[0:M, :], ps[0:M, 1024:2048])
        nc.vector.scalar_tensor_tensor(sq[0:M, 1024:2048], gyc[0:M, :],
                                       -1.0, gyc[0:M, :], OP.mult, OP.mult)
        # b = (sq_x - 0.25) > -sq_y  <=>  gx^2 + gy^2 > 0.25
        nc.vector.scalar_tensor_tensor(m[0:M, 0:512], sq[0:M, 0:512], -0.25,
                                       sq[0:M, 1024:1536], OP.add, OP.is_gt)
        nc.vector.scalar_tensor_tensor(m[0:M, 512:1024], sq[0:M, 512:1024],
                                       -0.25, sq[0:M, 1536:2048],
                                       OP.add, OP.is_gt)
        ind = prod_pool.tile([128, 512], F16, tag="ind")
        nc.vector.scalar_tensor_tensor(ind[0:M, :], m[0:M, 0:512], 0.0,
                                       m[0:M, 512:1024], OP.bypass,
                                       OP.not_equal,
                                       accum_out=counts_sb[0:M,
                                                           set_idx:set_idx+1])


def _build_nc(repeat: int = 1, loop_reps: int = 0):
    nc = bacc.Bacc("TRN2", target_bir_lowering=False, debug=False,
                   num_devices=N_CORES,
                   num_swdge_queues=int(os.environ.get("BASS_SWQ", "1")))
    logits = nc.declare_dram_parameter("logits", [BPC, C, H, W], F32,
                                       isOutput=False)
    targets = nc.declare_dram_parameter("targets", [BPC, C, H, W], F32,
                                        isOutput=False)
    bands = nc.declare_dram_parameter("bands", [128, BANDW], F16,
                                      isOutput=False)
    counts = nc.declare_dram_parameter("counts", [128, N_SETS], F32,
                                       isOutput=True)

    with tile.TileContext(nc) as tc:
        from contextlib import ExitStack
        with ExitStack() as ctx:
            consts = ctx.enter_context(tc.tile_pool(name="consts", bufs=1))
            psum_pool = ctx.enter_context(
                tc.tile_pool(name="psum", bufs=2, space="PSUM"))
            sq_pool = ctx.enter_context(tc.tile_pool(name="sqp", bufs=3))
            m_pool = ctx.enter_context(tc.tile_pool(name="mp", bufs=3))
            prod_pool = ctx.enter_context(tc.tile_pool(name="prodp", bufs=3))
            nsq_pool = ctx.enter_context(tc.tile_pool(name="nsqp", bufs=3))

            wsb = consts.tile([128, BANDW], F16)
            nc.sync.dma_start(out=wsb, in_=bands[:, :])
            bias0 = consts.tile([128, 1], F32)
            nc.vector.memset(bias0, 0.0)
            counts_sb = consts.tile([128, N_SETS], F32)
            nc.vector.memset(counts_sb, 0.0)

            # image-wide input buffers, manually rotated:
            #   lt: logits fp32 [128, 4 chunks, 514]
            #   xt: pred|tgt fp16 [128, 4 chunks, 2 srcs, 514]
            # zero pad cols (0 and 513 of each slot) persist across iters
            NBUF = 3
            lts, xts = [], []
            for i in range(NBUF):
                lt = consts.tile([128, 4, CW], F32, name=f"lt{i}")
                xt = consts.tile([128, 4, 2, CW], F16, name=f"xt{i}")
                nc.vector.memset(xt[:, :, :, 0:1], 0.0)
                nc.vector.memset(xt[:, :, :, 513:514], 0.0)
                # block0 input rows cover only 127 partitions; zero row 127
                # of chunk 0 so sigmoid reads defined data there
                nc.vector.memset(lt[96:128, 0, :], 0.0)
                nc.vector.memset(xt[96:128, 0, :, :], 0.0)
                lts.append(lt)
                xts.append(xt)
            # leftover combined tiles
            lt_l = consts.tile([128, CW], F32, name="lt_l")
            xt_l = consts.tile([128, 2, CW], F16, name="xt_l")
            nc.vector.memset(xt_l[:, :, 0:1], 0.0)
            nc.vector.memset(xt_l[:, :, 513:514], 0.0)

            from contextlib import nullcontext
            loop_cm = (tc.For_i(0, loop_reps, 1) if loop_reps
                       else nullcontext())
            with loop_cm:
              for rep in range(repeat):
                set_idx = 0
                for img in range(N_IMG):
                    b, c = divmod(img, C)
                    j = (rep * N_IMG + img) % NBUF
                    lt, xt = lts[j], xts[j]
                    # logits fp32 via HWDGE into lt; targets via SWDGE
                    # cast-DMA straight to fp16 into xt slot 1.
                    # 2 DMAs per tensor: chunk0 [127,512], then chunks 1-3
                    # in one DMA whose source re-reads the 2-row halo
                    # (overlapping 128-row windows, stride 126 rows)
                    limg = logits[b, c]
                    nc.sync.dma_start(out=lt[0:127, 0, 1:513],
                                      in_=limg[0:127, :])
                    lsrc3 = bass.AP(
                        tensor=limg.tensor,
                        offset=limg.offset + 125 * W,
                        ap=[[W, 128], [126 * W, 3], [1, W]])
                    nc.sync.dma_start(out=lt[:, 1:4, 1:513], in_=lsrc3)
                    timg = targets[b, c]
                    nc.gpsimd.dma_start(out=xt[0:127, 0, 1, 1:513],
                                        in_=timg[0:127, :])
                    tsrc3 = bass.AP(
                        tensor=timg.tensor,
                        offset=timg.offset + 125 * W,
                        ap=[[W, 128], [126 * W, 3], [1, W]])
                    nc.gpsimd.dma_start(out=xt[:, 1:4, 1, 1:513], in_=tsrc3)
                    # sigmoid split per DMA piece: chunk0 can be processed
                    # while the chunks1-3 DMA is still in flight
                    if _PIECES != "dma":
                        nc.scalar.activation(xt[0:127, 0, 0, 1:513],
                                             lt[0:127, 0, 1:513],
                                             AF.Sigmoid,
                                             bias=bias0[0:127, 0:1])
                        nc.scalar.activation(xt[:, 1:4, 0, 1:513],
                                             lt[:, 1:4, 1:513],
                                             AF.Sigmoid, bias=bias0[:, 0:1])

                    for blk, (ostart, istart, K, M) in enumerate(MAIN_BLOCKS):
                        _emit_tail(nc, wsb, counts_sb, set_idx, xt, blk, K, M,
                                   'b0' if ostart == 0 else 'int',
                                   psum_pool, sq_pool, m_pool, prod_pool,
                                   nsq_pool, bias0)
                        set_idx += 1

                # leftover rows of all images, block-diagonal combined set
                # (one DMA per tensor: src [8 imgs, 9 rows, 512] -> 72 parts)
                lsrc_left = bass.AP(
                    tensor=logits[0, 0].tensor,
                    offset=logits[0, 0].offset + LEFT_IN * W,
                    ap=[[H * W, N_IMG], [W, LEFT_K], [1, W]])
                nc.sync.dma_start(out=lt_l[0:N_IMG*LEFT_K, 1:513],
                                  in_=lsrc_left)
                tsrc_left = bass.AP(
                    tensor=targets[0, 0].tensor,
                    offset=targets[0, 0].offset + LEFT_IN * W,
                    ap=[[H * W, N_IMG], [W, LEFT_K], [1, W]])
                nc.gpsimd.dma_start(out=xt_l[0:N_IMG*LEFT_K, 1, 1:513],
                                    in_=tsrc_left)
                KL, ML = LEFT_K * N_IMG, LEFT_M * N_IMG
                if _PIECES != "dma":
                    nc.scalar.activation(xt_l[0:KL, 0, 1:513],
                                         lt_l[0:KL, 1:513],
                                         AF.Sigmoid, bias=bias0[0:KL, 0:1])
                # view [128, 1, 2, CW] so _emit_tail's indexing works
                xt_l4 = xt_l.rearrange("p (n s) w -> p n s w", n=1)
                _emit_tail(nc, wsb, counts_sb, set_idx, xt_l4, 0,
                           KL, ML, 'left', psum_pool, sq_pool, m_pool,
                           prod_pool, nsq_pool, bias0)

            nc.sync.dma_start(out=counts[:, :], in_=counts_sb)
    nc.compile()
    return nc


_NC = None
LAST_RESULT = None


def kernel(logits: np.ndarray, targets: np.ndarray) -> np.ndarray:
    global _NC, LAST_RESULT
    if _NC is None:
        _NC = _build_nc()

    logits = np.ascontiguousarray(logits, dtype=np.float32)
    targets = np.ascontiguousarray(targets, dtype=np.float32)
    in_maps = []
    for c in range(N_CORES):
        in_maps.append({
            "logits": logits[c*BPC:(c+1)*BPC],
            "targets": targets[c*BPC:(c+1)*BPC],
            "bands": _BANDS.astype(np.float16),
        })
    res = run_bass_kernel_spmd(
        _NC, in_maps, list(range(N_CORES)),
        trace=bool(os.environ.get("BASS_TRACE_KERNEL")),
    )
    LAST_RESULT = res
    total_xor = 0.0
    for r in res.results:
        total_xor += float(np.asarray(r["counts"], dtype=np.float64).sum())
    loss = 100.0 * total_xor / float(B * C * H * W)
    return np.float32(loss)


# revision 8
# speedup vs baseline: 1.3854x; 1.3854x over previous
"""Trainium2 Bass kernel for BoundaryLoss.

loss = mean over pixels of BCE(pred_b, tgt_b) where pred_b/tgt_b are 0/1
Sobel-boundary maps of sigmoid(logits) / targets. Since both maps are
binary, the clamped BCE reduces exactly to 100 * mean(pred_b XOR tgt_b).

Strategy (pure data parallel over batch, 2 samples -> 8 images per core):
  - logits loaded fp32 (HWDGE); sigmoid fp32->fp16 on ScalarE
  - targets loaded via SWDGE cast-DMA fp32->fp16 (no engine cost)
  - pred/tgt fp16 chunks interleaved in one tile so each block-set shares
    a single 4-bank PSUM tile [gx_p|gx_t|gy_p|gy_t]
  - full 2D Sobel conv on TensorE fp16: vertical band-matrix matmuls with
    column-shifted moving operands accumulating in PSUM (gx: 2 matmuls,
    gy: 3 matmuls, per src) per 126-row block
  - ScalarE squares all 4 quadrants in one instruction (2048 wide)
  - VectorE: everything via plain scalar_tensor_tensor (the only fast DVE
    op on HW: ~281ns/512cols; tensor_tensor and accum_out ops measured
    4-10x slower): margins, product, and a fused in-place spatial
    accumulator acc += (prod < 0). No accum_out in the hot loop; the
    [128, 512] accumulator map is DMA'd out and summed on host.
Each 512-row image = 4 main blocks of 126 output rows (input 128 rows
incl. 1-row halo) + an 8-row leftover; leftovers of all 8 images are
batched into one block-diagonal matmul set.
"""
import os
import numpy as np

import concourse.bass as bass
import concourse.tile as tile
from concourse import bacc, mybir
from concourse.bass_utils import run_bass_kernel_spmd

F32 = mybir.dt.float32
F16 = mybir.dt.float16
AF = mybir.ActivationFunctionType
OP = mybir.AluOpType

B, C, H, W = 16, 4, 512, 512
N_CORES = 8
BPC = B // N_CORES          # batch entries per core
N_IMG = BPC * C             # images per core
MAIN_BLOCKS = [(0, 0, 127, 126), (126, 125, 128, 126),
               (252, 251, 128, 126), (378, 377, 128, 126)]
LEFT_IN, LEFT_OUT, LEFT_K, LEFT_M = 503, 504, 9, 8
N_SETS = N_IMG * len(MAIN_BLOCKS) + 1
CW = 514                                 # block chunk width incl pad cols


# ---------------------------------------------------------------- bands
def _band_pair(in_rows, out_rows):
    K, M = len(in_rows), len(out_rows)
    vs = np.zeros((K, M), np.float32)
    vd = np.zeros((K, M), np.float32)
    for k, ir in enumerate(in_rows):
        for m, orow in enumerate(out_rows):
            d = ir - orow
            if d == 0:
                vs[k, m] = 2.0
            elif abs(d) == 1:
                vs[k, m] = 1.0
                vd[k, m] = float(d)
    return vs, vd


def _build_band_tensor():
    """Stack all band matrices into one [128, total_cols] array.
    offsets[(key, wname)] = (col, K, M)."""
    specs = {}
    specs['b0'] = _band_pair(range(0, 127), range(0, 126))
    specs['int'] = _band_pair(range(125, 253), range(126, 252))
    K, M = LEFT_K * N_IMG, LEFT_M * N_IMG
    vs = np.zeros((K, M), np.float32)
    vd = np.zeros((K, M), np.float32)
    svs, svd = _band_pair(range(LEFT_IN, 512), range(LEFT_OUT, 512))
    for i in range(N_IMG):
        vs[i*LEFT_K:(i+1)*LEFT_K, i*LEFT_M:(i+1)*LEFT_M] = svs
        vd[i*LEFT_K:(i+1)*LEFT_K, i*LEFT_M:(i+1)*LEFT_M] = svd
    specs['left'] = (vs, vd)

    cols = []
    offsets = {}
    col = 0
    for key, (vs, vd) in specs.items():
        for wname, wmat in (("vs", vs), ("vsn", -vs), ("vd2", 2.0*vd), ("vd", vd)):
            K, M = wmat.shape
            buf = np.zeros((128, M), np.float32)
            buf[:K, :] = wmat
            cols.append(buf)
            offsets[(key, wname)] = (col, K, M)
            col += M
    return np.concatenate(cols, axis=1), offsets


_BANDS, _BOFF = _build_band_tensor()
BANDW = _BANDS.shape[1]


# ---------------------------------------------------------------- kernel
_PIECES = os.environ.get("BASS_KERNEL_PIECES", "full")


def _emit_tail(nc, wsb, acc, set_idx, src, col0, K, M,
               band_key, psum_pool, sq_pool, m_pool, prod_pool, bias0):
    """Matmuls + squares + margins + xor accumulate for one block-set.
    src: fp16 tile viewed [128, nchunk, 2, 514]; col0 selects the chunk.
    Data at cols [1, 513) of each (chunk, src) slot, zero pads at 0/513."""
    if _PIECES in ("io", "dma"):
        return

    def wap(wname):
        col, kk, mm = _BOFF[(band_key, wname)]
        assert kk == K and mm == M
        return wsb[0:K, col:col + M]

    ps = psum_pool.tile([128, 2048], F32, tag="ps")
    # quadrant layout: [gx_p | gx_t | gy_p | gy_t], 512 fp32 = 1 bank each
    for half in (0, 1):
        mv = lambda s: src[0:K, col0, half, s:s+512]
        gx = ps[0:M, half*512:(half+1)*512]
        nc.tensor.matmul(gx, wap("vs"), mv(2), start=True, stop=False)
        nc.tensor.matmul(gx, wap("vsn"), mv(0), start=False, stop=True)
    for half in (0, 1):
        mv = lambda s: src[0:K, col0, half, s:s+512]
        gy = ps[0:M, 1024+half*512:1024+(half+1)*512]
        nc.tensor.matmul(gy, wap("vd2"), mv(1), start=True, stop=False)
        nc.tensor.matmul(gy, wap("vd"), mv(0), start=False, stop=False)
        nc.tensor.matmul(gy, wap("vd"), mv(2), start=False, stop=True)

    if _PIECES == "conv":
        return

    sq = sq_pool.tile([128, 2048], F16, tag="sq")
    nc.scalar.activation(sq[0:M, :], ps[0:M, :], AF.Square,
                         bias=bias0[0:M, 0:1])
    m = m_pool.tile([128, 1024], F16, tag="m")
    # m = (sq_x - 0.25) + sq_y
    nc.vector.scalar_tensor_tensor(m[0:M, 0:512], sq[0:M, 0:512], -0.25,
                                   sq[0:M, 1024:1536], OP.add, OP.add)
    nc.vector.scalar_tensor_tensor(m[0:M, 512:1024], sq[0:M, 512:1024],
                                   -0.25, sq[0:M, 1536:2048],
                                   OP.add, OP.add)
    prod = prod_pool.tile([128, 512], F16, tag="prod")
    nc.vector.scalar_tensor_tensor(prod[0:M, :], m[0:M, 0:512], 0.0,
                                   m[0:M, 512:1024], OP.add, OP.mult)
    # acc += (prod < 0), fused in-place
    nc.vector.scalar_tensor_tensor(acc[0:M, :], prod[0:M, :], 0.0,
                                   acc[0:M, :], OP.is_lt, OP.add)


def _build_nc(repeat: int = 1, loop_reps: int = 0):
    nc = bacc.Bacc("TRN2", target_bir_lowering=False, debug=False,
                   num_devices=N_CORES,
                   num_swdge_queues=int(os.environ.get("BASS_SWQ", "1")))
    logits = nc.declare_dram_parameter("logits", [BPC, C, H, W], F32,
                                       isOutput=False)
    targets = nc.declare_dram_parameter("targets", [BPC, C, H, W], F32,
                                        isOutput=False)
    bands = nc.declare_dram_parameter("bands", [128, BANDW], F16,
                                      isOutput=False)
    counts = nc.declare_dram_parameter("counts", [128, 512], F16,
                                       isOutput=True)

    with tile.TileContext(nc) as tc:
        from contextlib import ExitStack
        with ExitStack() as ctx:
            consts = ctx.enter_context(tc.tile_pool(name="consts", bufs=1))
            psum_pool = ctx.enter_context(
                tc.tile_pool(name="psum", bufs=2, space="PSUM"))
            sq_pool = ctx.enter_context(tc.tile_pool(name="sqp", bufs=3))
            m_pool = ctx.enter_context(tc.tile_pool(name="mp", bufs=3))
            prod_pool = ctx.enter_context(tc.tile_pool(name="prodp", bufs=3))

            wsb = consts.tile([128, BANDW], F16)
            nc.sync.dma_start(out=wsb, in_=bands[:, :])
            bias0 = consts.tile([128, 1], F32)
            nc.vector.memset(bias0, 0.0)
            acc = consts.tile([128, 512], F16, name="acc")
            nc.vector.memset(acc, 0.0)

            # image-wide input buffers, manually rotated:
            #   lt: logits fp32 [128, 4 chunks, 514]
            #   xt: pred|tgt fp16 [128, 4 chunks, 2 srcs, 514]
            # zero pad cols (0 and 513 of each slot) persist across iters
            NBUF = 3
            lts, xts = [], []
            for i in range(NBUF):
                lt = consts.tile([128, 4, CW], F32, name=f"lt{i}")
                xt = consts.tile([128, 4, 2, CW], F16, name=f"xt{i}")
                nc.vector.memset(xt[:, :, :, 0:1], 0.0)
                nc.vector.memset(xt[:, :, :, 513:514], 0.0)
                # block0 input rows cover only 127 partitions; zero row 127
                # of chunk 0 so sigmoid reads defined data there
                nc.vector.memset(lt[96:128, 0, :], 0.0)
                nc.vector.memset(xt[96:128, 0, :, :], 0.0)
                lts.append(lt)
                xts.append(xt)
            # leftover combined tiles
            lt_l = consts.tile([128, CW], F32, name="lt_l")
            xt_l = consts.tile([128, 2, CW], F16, name="xt_l")
            nc.vector.memset(xt_l[:, :, 0:1], 0.0)
            nc.vector.memset(xt_l[:, :, 513:514], 0.0)

            from contextlib import nullcontext
            loop_cm = (tc.For_i(0, loop_reps, 1) if loop_reps
                       else nullcontext())
            with loop_cm:
              for rep in range(repeat):
                set_idx = 0
                for img in range(N_IMG):
                    b, c = divmod(img, C)
                    j = (rep * N_IMG + img) % NBUF
                    lt, xt = lts[j], xts[j]
                    # logits fp32 via HWDGE into lt; targets via SWDGE
                    # cast-DMA straight to fp16 into xt slot 1.
                    # 2 DMAs per tensor: chunk0 [127,512], then chunks 1-3
                    # in one DMA whose source re-reads the 2-row halo
                    # (overlapping 128-row windows, stride 126 rows)
                    limg = logits[b, c]
                    nc.sync.dma_start(out=lt[0:127, 0, 1:513],
                                      in_=limg[0:127, :])
                    lsrc3 = bass.AP(
                        tensor=limg.tensor,
                        offset=limg.offset + 125 * W,
                        ap=[[W, 128], [126 * W, 3], [1, W]])
                    nc.sync.dma_start(out=lt[:, 1:4, 1:513], in_=lsrc3)
                    timg = targets[b, c]
                    nc.gpsimd.dma_start(out=xt[0:127, 0, 1, 1:513],
                                        in_=timg[0:127, :])
                    tsrc3 = bass.AP(
                        tensor=timg.tensor,
                        offset=timg.offset + 125 * W,
                        ap=[[W, 128], [126 * W, 3], [1, W]])
                    nc.gpsimd.dma_start(out=xt[:, 1:4, 1, 1:513], in_=tsrc3)
                    # sigmoid split per DMA piece: chunk0 can be processed
                    # while the chunks1-3 DMA is still in flight
                    if _PIECES != "dma":
                        nc.scalar.activation(xt[0:127, 0, 0, 1:513],
                                             lt[0:127, 0, 1:513],
                                             AF.Sigmoid,
                                             bias=bias0[0:127, 0:1])
                        nc.scalar.activation(xt[:, 1:4, 0, 1:513],
                                             lt[:, 1:4, 1:513],
                                             AF.Sigmoid, bias=bias0[:, 0:1])

                    for blk, (ostart, istart, K, M) in enumerate(MAIN_BLOCKS):
                        _emit_tail(nc, wsb, acc, set_idx, xt, blk, K, M,
                                   'b0' if ostart == 0 else 'int',
                                   psum_pool, sq_pool, m_pool, prod_pool,
                                   bias0)
                        set_idx += 1

                # leftover rows of all images, block-diagonal combined set
                # (one DMA per tensor: src [8 imgs, 9 rows, 512] -> 72 parts)
                lsrc_left = bass.AP(
                    tensor=logits[0, 0].tensor,
                    offset=logits[0, 0].offset + LEFT_IN * W,
                    ap=[[H * W, N_IMG], [W, LEFT_K], [1, W]])
                nc.sync.dma_start(out=lt_l[0:N_IMG*LEFT_K, 1:513],
                                  in_=lsrc_left)
                tsrc_left = bass.AP(
                    tensor=targets[0, 0].tensor,
                    offset=targets[0, 0].offset + LEFT_IN * W,
                    ap=[[H * W, N_IMG], [W, LEFT_K], [1, W]])
                nc.gpsimd.dma_start(out=xt_l[0:N_IMG*LEFT_K, 1, 1:513],
                                    in_=tsrc_left)
                KL, ML = LEFT_K * N_IMG, LEFT_M * N_IMG
                if _PIECES != "dma":
                    nc.scalar.activation(xt_l[0:KL, 0, 1:513],
                                         lt_l[0:KL, 1:513],
                                         AF.Sigmoid, bias=bias0[0:KL, 0:1])
                # view [128, 1, 2, CW] so _emit_tail's indexing works
                xt_l4 = xt_l.rearrange("p (n s) w -> p n s w", n=1)
                _emit_tail(nc, wsb, acc, set_idx, xt_l4, 0,
                           KL, ML, 'left', psum_pool, sq_pool, m_pool,
                           prod_pool, bias0)

            nc.sync.dma_start(out=counts[:, :], in_=acc)
    nc.compile()
    return nc


_NC = None
LAST_RESULT = None


def kernel(logits: np.ndarray, targets: np.ndarray) -> np.ndarray:
    global _NC, LAST_RESULT
    if _NC is None:
        _NC = _build_nc()

    logits = np.ascontiguousarray(logits, dtype=np.float32)
    targets = np.ascontiguousarray(targets, dtype=np.float32)
    in_maps = []
    for c in range(N_CORES):
        in_maps.append({
            "logits": logits[c*BPC:(c+1)*BPC],
            "targets": targets[c*BPC:(c+1)*BPC],
            "bands": _BANDS.astype(np.float16),
        })
    res = run_bass_kernel_spmd(
        _NC, in_maps, list(range(N_CORES)),
        trace=bool(os.environ.get("BASS_TRACE_KERNEL")),
    )
    LAST_RESULT = res
    total_xor = 0.0
    for r in res.results:
        total_xor += float(np.asarray(r["counts"], dtype=np.float64).sum())
    loss = 100.0 * total_xor / float(B * C * H * W)
    return np.float32(loss)


# revision 16
# speedup vs baseline: 2.2003x; 1.5882x over previous
"""Trainium2 Bass kernel for BoundaryLoss, V5: big-DMA restructure.

loss = 100 * mean(pred_b XOR tgt_b), boundary maps via Sobel magnitude
thresholding (see kernel docstring history).

V5 layout: each 512-row image is 4 NON-overlapping 128-row windows
(chunks). One DMA per tensor per image-PAIR brings 8 windows = 2.1MB in
a single sequential HBM read. Window-interior output rows (127+126+126
+127 = 506 per image) are produced by per-chunk band matmuls; the 6
boundary rows per image that straddle window edges are patched by one
extra block-diagonal set fed by small re-read DMAs.

Rings: logits fp32 alternates the two HWDGE rings (nc.sync / nc.scalar);
targets cast fp32->fp16 on the SWDGE ring (nc.gpsimd).

Evacuation (per set): ScalarE Square of the 4-bank PSUM group (2x rate);
VectorE margins/product/accumulate via plain scalar_tensor_tensor only,
with an in-place fp16 accumulator map summed on host.
"""
import os
import numpy as np
import ml_dtypes

import concourse.bass as bass
import concourse.tile as tile
from concourse import bacc, mybir
from concourse.bass_utils import run_bass_kernel_spmd

F32 = mybir.dt.float32
F16 = mybir.dt.float16
F8 = mybir.dt.float8e4
AF = mybir.ActivationFunctionType
OP = mybir.AluOpType
PM = mybir.MatmulPerfMode

B, C, H, W = 16, 4, 512, 512
N_CORES = 8
BPC = B // N_CORES
N_IMG = BPC * C             # 8 images per core
N_PAIR = N_IMG // 2         # 4 pairs
N_SETS = N_IMG * 4 + 1
CW = 514

# (band_key, M) per chunk-in-image
CHUNK_BANDS = [('b0', 127), ('int', 126), ('int', 126), ('b3', 127)]
BND_K, BND_M = 96, 48       # boundary set: 3 boundaries x 8 imgs x (4 in/2 out)


def _band_pair(in_rows, out_rows):
    K, M = len(in_rows), len(out_rows)
    vs = np.zeros((K, M), np.float32)
    vd = np.zeros((K, M), np.float32)
    for k, ir in enumerate(in_rows):
        for m, orow in enumerate(out_rows):
            d = ir - orow
            if d == 0:
                vs[k, m] = 2.0
            elif abs(d) == 1:
                vs[k, m] = 1.0
                vd[k, m] = float(d)
    return vs, vd


def _build_band_tensor():
    specs = {}
    specs['b0'] = _band_pair(range(0, 128), range(0, 127))
    specs['int'] = _band_pair(range(0, 128), range(1, 127))
    specs['b3'] = _band_pair(range(0, 128), range(1, 128))
    # boundary set: partition p = (k-1)*32 + img*4 + r, out col 2*g + {0,1}
    # with g = (k-1)*8 + img; per-group band rows 128k-2..128k+1 ->
    # out rows 128k-1..128k (absolute), same [4,2] block for every k.
    svs, svd = _band_pair(range(126, 130), range(127, 129))
    vs = np.zeros((BND_K, BND_M), np.float32)
    vd = np.zeros((BND_K, BND_M), np.float32)
    for g in range(24):
        vs[g*4:(g+1)*4, g*2:(g+1)*2] = svs
        vd[g*4:(g+1)*4, g*2:(g+1)*2] = svd
    specs['bnd'] = (vs, vd)

    cols = []
    offsets = {}
    col = 0
    for key, (vs, vd) in specs.items():
        K, M = vs.shape
        Mp = 48 if key == 'bnd' else 128   # DR out-partition pad (step%16==0)
        drgx = np.zeros((128, 2, Mp), np.float32)
        drgx[:K, 0, :M] = -vs       # pairs with x_{-1}
        drgx[:K, 1, :M] = vs        # pairs with x_{+1}
        drgy = np.zeros((128, 2, Mp), np.float32)
        drgy[:K, 0, :M] = vd
        drgy[:K, 1, :M] = vd
        vd2 = np.zeros((128, M), np.float32)
        vd2[:K] = 2.0 * vd
        for wname, arr, mm in (("drgx", drgx.reshape(128, 2 * Mp), Mp),
                               ("drgy", drgy.reshape(128, 2 * Mp), Mp),
                               ("vd2", vd2, M)):
            cols.append(arr)
            offsets[(key, wname)] = (col, K, mm)
            col += arr.shape[1]
    return np.concatenate(cols, axis=1), offsets


_BANDS, _BOFF = _build_band_tensor()
BANDW = _BANDS.shape[1]
_BANDS_IO = _BANDS.astype(ml_dtypes.float8_e4m3)

_PIECES = os.environ.get("BASS_KERNEL_PIECES", "full")


def _emit_tail(nc, wsb, acc, src, col0, K, M, band_key,
               psum_pool, sq_pool, m_pool, prod_pool, bias0):
    """Matmuls + squares + margins + xor accumulate for one block-set.
    src: fp16 tile viewed [128, nchunk, 2, 514]; col0 selects the chunk."""
    if _PIECES in ("io", "dma"):
        return

    def wap(wname):
        col, kk, mm = _BOFF[(band_key, wname)]
        assert kk == K and mm == M
        return wsb[0:K, col:col + M]

    def wap_dr(wname):
        col, kk, mp = _BOFF[(band_key, wname)]
        assert kk == K
        return (wsb[0:K, col:col + 2 * mp]
                .rearrange("k (i m) -> k i m", i=2), mp)

    def rhs_dr(half):
        base = src[0:K, col0, half, 0:514]
        return bass.AP(tensor=base.tensor, offset=base.offset,
                       ap=[list(base.ap[0]), [2, 2], [1, 512]])

    ps = psum_pool.tile([128, 2048], F32, tag="ps")
    wgx, mpx = wap_dr("drgx")
    for half in (0, 1):
        nc.tensor.matmul(ps[0:mpx, half*512:(half+1)*512], wgx, rhs_dr(half),
                         start=True, stop=True, perf_mode=PM.DoubleRow)
    wgy, mpy = wap_dr("drgy")
    for half in (0, 1):
        nc.tensor.matmul(ps[0:mpy, 1024+half*512:1024+(half+1)*512], wgy,
                         rhs_dr(half), start=True, stop=False,
                         perf_mode=PM.DoubleRow)
    for half in (0, 1):
        nc.tensor.matmul(ps[0:M, 1024+half*512:1024+(half+1)*512],
                         wap("vd2"), src[0:K, col0, half, 1:513],
                         start=False, stop=True)

    if _PIECES == "conv":
        return

    sq = sq_pool.tile([128, 2048], F16, tag="sq")
    nc.scalar.activation(sq[0:M, :], ps[0:M, :], AF.Square,
                         bias=bias0[0:M, 0:1])
    m = m_pool.tile([128, 1024], F16, tag="m")
    nc.vector.scalar_tensor_tensor(m[0:M, :], sq[0:M, 0:1024], -0.25,
                                   sq[0:M, 1024:2048], OP.add, OP.add)
    prod = prod_pool.tile([128, 512], F16, tag="prod")
    nc.vector.scalar_tensor_tensor(prod[0:M, :], m[0:M, 0:512], 0.0,
                                   m[0:M, 512:1024], OP.add, OP.mult)
    nc.vector.scalar_tensor_tensor(acc[0:M, :], prod[0:M, :], 0.0,
                                   acc[0:M, :], OP.is_lt, OP.add)


def _build_nc(repeat: int = 1, loop_reps: int = 0):
    nc = bacc.Bacc("TRN2", target_bir_lowering=False, debug=False,
                   num_devices=N_CORES,
                   num_swdge_queues=int(os.environ.get("BASS_SWQ", "1")))
    logits = nc.declare_dram_parameter("logits", [BPC, C, H, W], F32,
                                       isOutput=False)
    targets = nc.declare_dram_parameter("targets", [BPC, C, H, W], F32,
                                        isOutput=False)
    bands = nc.declare_dram_parameter("bands", [128, BANDW], F8,
                                      isOutput=False)
    counts = nc.declare_dram_parameter("counts", [128, 512], F16,
                                       isOutput=True)

    with tile.TileContext(nc) as tc:
        from contextlib import ExitStack
        with ExitStack() as ctx:
            consts = ctx.enter_context(tc.tile_pool(name="consts", bufs=1))
            psum_pool = ctx.enter_context(
                tc.tile_pool(name="psum", bufs=2, space="PSUM"))
            sq_pool = ctx.enter_context(tc.tile_pool(name="sqp", bufs=3))
            m_pool = ctx.enter_context(tc.tile_pool(name="mp", bufs=3))
            prod_pool = ctx.enter_context(tc.tile_pool(name="prodp", bufs=3))

            wsb = consts.tile([128, BANDW], F8)
            nc.sync.dma_start(out=wsb, in_=bands[:, :])
            bias0 = consts.tile([128, 1], F32)
            nc.vector.memset(bias0, 0.0)
            acc = consts.tile([128, 512], F16, name="acc")
            nc.vector.memset(acc, 0.0)

            NBUF = 4
            lts, xts = [], []
            for i in range(NBUF):
                lt = consts.tile([128, 8, CW], F32, name=f"lt{i}")
                xt = consts.tile([128, 8, 2, CW], F8, name=f"xt{i}")
                nc.vector.memset(xt[:, :, :, 0:1], 0.0)
                nc.vector.memset(xt[:, :, :, 513:514], 0.0)
                lts.append(lt)
                xts.append(xt)
            # boundary tiles
            blt = consts.tile([128, CW], F32, name="blt")
            bxt = consts.tile([128, 2, CW], F8, name="bxt")
            nc.vector.memset(bxt[:, :, 0:1], 0.0)
            nc.vector.memset(bxt[:, :, 513:514], 0.0)

            lflat = logits[0, 0]     # base AP for flattened image offsets
            tflat = targets[0, 0]

            from contextlib import nullcontext
            loop_cm = (tc.For_i(0, loop_reps, 1) if loop_reps
                       else nullcontext())
            with loop_cm:
              for rep in range(repeat):
                # boundary-patch DMAs (small; issued up front)
                for k in (() if os.environ.get("BASS_NO_BND") else (1, 2, 3)):
                    off = (128 * k - 2) * W
                    lsrc = bass.AP(tensor=lflat.tensor,
                                   offset=lflat.offset + off,
                                   ap=[[H * W, N_IMG], [W, 4], [1, W]])
                    nc.sync.dma_start(out=blt[(k-1)*32:k*32, 1:513],
                                      in_=lsrc)
                    tsrc = bass.AP(tensor=tflat.tensor,
                                   offset=tflat.offset + off,
                                   ap=[[H * W, N_IMG], [W, 4], [1, W]])
                    nc.gpsimd.dma_start(out=bxt[(k-1)*32:k*32, 1, 1:513],
                                        in_=tsrc)
                if _PIECES != "dma" and not os.environ.get("BASS_NO_BND"):
                    nc.scalar.activation(bxt[0:BND_K, 0, 1:513],
                                         blt[0:BND_K, 1:513],
                                         AF.Sigmoid, bias=bias0[0:BND_K, 0:1])

                for pair in range(N_PAIR):
                    j = (rep * N_PAIR + pair) % NBUF
                    lt, xt = lts[j], xts[j]
                    poff = (2 * pair) * H * W
                    lsrc = bass.AP(tensor=lflat.tensor,
                                   offset=lflat.offset + poff,
                                   ap=[[W, 128], [128 * W, 8], [1, W]])
                    leng = nc.sync if pair % 2 == 0 else nc.scalar
                    leng.dma_start(out=lt[:, :, 1:513], in_=lsrc)
                    tsrc = bass.AP(tensor=tflat.tensor,
                                   offset=tflat.offset + poff,
                                   ap=[[W, 128], [128 * W, 8], [1, W]])
                    nc.gpsimd.dma_start(out=xt[:, :, 1, 1:513], in_=tsrc)
                    if _PIECES != "dma":
                        nc.scalar.activation(xt[:, 0:4, 0, 1:513],
                                             lt[:, 0:4, 1:513],
                                             AF.Sigmoid, bias=bias0[:, 0:1])
                        nc.scalar.activation(xt[:, 4:8, 0, 1:513],
                                             lt[:, 4:8, 1:513],
                                             AF.Sigmoid, bias=bias0[:, 0:1])
                    for chunk in range(8):
                        key, M = CHUNK_BANDS[chunk % 4]
                        _emit_tail(nc, wsb, acc, xt, chunk, 128, M, key,
                                   psum_pool, sq_pool, m_pool, prod_pool,
                                   bias0)

                # boundary-patch set
                if not os.environ.get("BASS_NO_BND"):
                    bxt4 = bxt.rearrange("p (n s) w -> p n s w", n=1)
                    _emit_tail(nc, wsb, acc, bxt4, 0, BND_K, BND_M, 'bnd',
                               psum_pool, sq_pool, m_pool, prod_pool, bias0)

            nc.sync.dma_start(out=counts[:, :], in_=acc)
    nc.compile()
    return nc


_NC = None
LAST_RESULT = None


def kernel(logits: np.ndarray, targets: np.ndarray) -> np.ndarray:
    global _NC, LAST_RESULT
    if _NC is None:
        _NC = _build_nc()

    logits = np.ascontiguousarray(logits, dtype=np.float32)
    targets = np.ascontiguousarray(targets, dtype=np.float32)
    in_maps = []
    for c in range(N_CORES):
        in_maps.append({
            "logits": logits[c*BPC:(c+1)*BPC],
            "targets": targets[c*BPC:(c+1)*BPC],
            "bands": _BANDS_IO,
        })
    res = run_bass_kernel_spmd(
        _NC, in_maps, list(range(N_CORES)),
        trace=bool(os.environ.get("BASS_TRACE_KERNEL")),
    )
    LAST_RESULT = res
    total_xor = 0.0
    for r in res.results:
        total_xor += float(np.asarray(r["counts"], dtype=np.float64).sum())
    loss = 100.0 * total_xor / float(B * C * H * W)
    return np.float32(loss)
